# revision 4
# baseline (speedup 1.0000x reference)
"""Trainium2 Bass kernel for noisy-softmax multi-head attention (nn_BAttention).

Full inputs -> full outputs; internally shards (batch x head) across 8 NeuronCores
Megatron-style (2 heads per core, Q/K/V/O weights column/row-sharded).

Math: reference computes
    q = split(x@Wq)/sqrt(hd); k = split(x@Wk); v = split(x@Wv)
    logits_raw = q @ k^T            (per head)         <- output 2
    att = softmax(logits_raw + maskbias - gammaln + noise)
    out = merge(att @ v) @ Wo                          <- output 1
The gammaln shift is softmax-invariant. noise depends only on key(42), so the
host precomputes en = exp(noise + maskbias); the device computes
    P = exp(S) * en;  out_h = (P^T V)/Z with Z = sum_k P  (via ones columns).
Everything runs transposed (S^T = K^T-stationary x Q^T-moving) so no on-chip
transposes are needed except V (done on the PE).
"""
import os
import sys

import numpy as np

for _p in ("/opt/trn_rl_repo",):
    if os.path.isdir(_p) and _p not in sys.path:
        sys.path.append(_p)

B, S, H = 2, 2048, 1024
NH, HD = 16, 64
NCORES = 8
HPC = NH // NCORES          # heads per core
WEIBULL_K = 10.0
EPS = 1e-5
NKC = S // 128              # 16 k-strips
NQQ = S // 512              # 4 q-quarters

_PROGRAM = None


def _build_program():
    import concourse.bass as bass
    import concourse.bacc as bacc
    import concourse.mybir as mybir
    import concourse.tile as tile
    from concourse.masks import make_identity
    from contextlib import ExitStack

    f32, f16 = mybir.dt.float32, mybir.dt.float16
    EXP = mybir.ActivationFunctionType.Exp
    ds = bass.ds

    nc = bacc.Bacc("TRN2", target_bir_lowering=False, debug=False,
                   enable_asserts=True, num_devices=NCORES)

    xT_d = nc.dram_tensor("xT", [B, H, S], f16, kind="ExternalInput")
    wq_d = nc.dram_tensor("wq", [8, 128, 128], f16, kind="ExternalInput")
    wk_d = nc.dram_tensor("wk", [8, 128, 128], f16, kind="ExternalInput")
    wv_d = nc.dram_tensor("wv", [8, 128, 128], f16, kind="ExternalInput")
    wo_d = nc.dram_tensor("wo", [128, H], f16, kind="ExternalInput")
    # layouts: [b, k, qQ, h, qr] with q = qQ*512 + qr
    en_d = nc.dram_tensor("en", [B, S, NQQ, HPC, 512], f16, kind="ExternalInput")
    s16_d = nc.dram_tensor("s16", [B, S, NQQ, HPC, 512], f16, kind="ExternalOutput")
    outp_d = nc.dram_tensor("outp", [B, S, H], f16, kind="ExternalOutput")

    with tile.TileContext(nc) as tc, ExitStack() as ctx:
        const = ctx.enter_context(tc.tile_pool(name="const", bufs=1))
        wpool = ctx.enter_context(tc.tile_pool(name="wpool", bufs=1))
        xtp = ctx.enter_context(tc.tile_pool(name="xtp", bufs=1))
        qkv = ctx.enter_context(tc.tile_pool(name="qkv", bufs=1))
        vop = ctx.enter_context(tc.tile_pool(name="vop", bufs=2))
        enp = ctx.enter_context(tc.tile_pool(name="enp", bufs=6))
        pp = ctx.enter_context(tc.tile_pool(name="pp", bufs=4))
        s16p = ctx.enter_context(tc.tile_pool(name="s16p", bufs=6))
        rp = ctx.enter_context(tc.tile_pool(name="rp", bufs=2))
        otp = ctx.enter_context(tc.tile_pool(name="otp", bufs=1))
        osp = ctx.enter_context(tc.tile_pool(name="osp", bufs=3))
        psum = ctx.enter_context(tc.tile_pool(name="psum", bufs=2, space="PSUM"))

        copy_ctr = [0]

        def bal_copy(dst, srcap):
            # ~3/8 of PSUM->SBUF copies on ACT, rest on DVE (keeps both ~equal)
            i = copy_ctr[0]; copy_ctr[0] += 1
            if i % 8 < 3:
                nc.scalar.copy(dst, srcap)
            else:
                nc.vector.tensor_copy(dst, srcap)

        ident = const.tile([128, 128], f16, tag="ident")
        make_identity(nc, ident[:])

        w_sb = {}
        for nm, d in (("wq", wq_d), ("wk", wk_d), ("wv", wv_d)):
            t = wpool.tile([128, 8, 128], f16, tag=nm, name=nm + "_sb")
            for ch in range(8):
                nc.sync.dma_start(t[:, ch, :], d.ap()[ch])
            w_sb[nm] = t
        wo_sb = wpool.tile([128, H], f16, tag="wo")
        nc.sync.dma_start(wo_sb[:], wo_d.ap())

        QT, KT, OT = {}, {}, {}
        VO = {}  # VO[(b, h, kc)]
        for b in range(B):
            QT[b] = qkv.tile([128, S], f16, tag=f"qt{b}", name=f"qt{b}")
            KT[b] = qkv.tile([128, S], f16, tag=f"kt{b}", name=f"kt{b}")
            OT[b] = otp.tile([128, S], f16, tag=f"ot{b}", name=f"ot{b}")

        def qkv_pieces(b):
            """Generator of work-unit closures for batch-b QKV + V transpose."""
            xt = []

            def load_xt():
                for ch in range(8):
                    t = xtp.tile([128, S], f16, tag=f"xt{ch}", name=f"xt{b}_{ch}")
                    nc.sync.dma_start(t[:], xT_d.ap()[b, ds(ch * 128, 128), :])
                    xt.append(t)
            yield load_xt

            vt_box = []

            def proj(wname, dest, qd, destcol):
                def go():
                    ps = psum.tile([128, 1024], f32, tag="s",
                                   name=f"qkv_{b}_{wname}_{qd}")
                    for ch in range(8):
                        for nn in range(2):
                            nc.tensor.matmul(
                                ps[:, ds(nn * 512, 512)],
                                w_sb[wname][:, ch, :],
                                xt[ch][:, ds(qd * 1024 + nn * 512, 512)],
                                start=(ch == 0), stop=(ch == 7),
                            )
                    bal_copy(dest[:, ds(destcol, 1024)], ps[:])
                return go

            def mk_vt():
                vt = qkv.tile([128, S], f16, tag="vt", name=f"vt{b}", bufs=2)
                vt_box.append(vt)
            yield mk_vt
            for qd in range(2):
                yield proj("wq", QT[b], qd, qd * 1024)
            for qd in range(2):
                yield proj("wk", KT[b], qd, qd * 1024)

            def projv(qd):
                def go():
                    proj("wv", vt_box[0], qd, qd * 1024)()
                return go
            for qd in range(2):
                yield projv(qd)

            def vtrans(kc):
                def go():
                    vt = vt_box[0]
                    pst = psum.tile([128, 128], f16, tag="s", name=f"vtr{b}_{kc}")
                    nc.tensor.transpose(pst[:], vt[:, ds(kc * 128, 128)], ident[:])
                    vo0 = vop.tile([128, 128], f16, tag=f"vo0_{kc}",
                                   name=f"vo0_{b}_{kc}")
                    vo1 = vop.tile([128, 128], f16, tag=f"vo1_{kc}",
                                   name=f"vo1_{b}_{kc}")
                    nc.vector.tensor_copy(vo0[:, 0:64], pst[:, 0:64])
                    nc.gpsimd.memset(vo0[:, 64:128], 1.0)
                    nc.vector.tensor_copy(vo1[:, 64:128], pst[:, 64:128])
                    nc.gpsimd.memset(vo1[:, 0:64], 1.0)
                    VO[(b, 0, kc)] = vo0
                    VO[(b, 1, kc)] = vo1
                return go
            for kc in range(NKC):
                yield vtrans(kc)

        def final_pieces(b):
            """Generator of work-unit closures for batch-b output projection."""
            def fin(st):
                def go():
                    psf = psum.tile([128, 1024], f32, tag="s", name=f"fin_{b}_{st}")
                    for hh in range(2):
                        nc.tensor.matmul(
                            psf[:, ds(hh * 512, 512)],
                            OT[b][:, ds(st * 128, 128)],
                            wo_sb[:, ds(hh * 512, 512)],
                            start=True, stop=True,
                        )
                    ost = osp.tile([128, 1024], f16, tag="ost", name=f"ost_{b}_{st}")
                    bal_copy(ost[:], psf[:])
                    nc.sync.dma_start(outp_d.ap()[b, ds(st * 128, 128), :], ost[:])
                return go
            for st in range(S // 128):
                yield fin(st)

        def run_att(b, background):
            """Attention for batch b; pumps one background piece per strip."""
            for qh in range(2):
                u_ps = [
                    psum.tile([128, 1024], f32, tag="u", name=f"u{h}_{b}_{qh}")
                    for h in range(HPC)
                ]
                attv_prev = [None]

                def mk_attv(kc, p_t):
                    def go():
                        for h in range(HPC):
                            for qq in range(2):
                                nc.tensor.matmul(
                                    u_ps[h][:, ds(qq * 512, 512)],
                                    VO[(b, h, kc)][:],
                                    p_t[:, qq, h, :],
                                    start=(kc == 0), stop=(kc == NKC - 1),
                                )
                    return go

                for kc in range(NKC):
                    en_t = enp.tile([128, 2, HPC, 512], f16, tag="en",
                                    name=f"en_{b}_{qh}_{kc}")
                    nc.sync.dma_start(
                        en_t[:],
                        en_d.ap()[b, ds(kc * 128, 128), ds(qh * 2, 2), :, :])
                    p_t = pp.tile([128, 2, HPC, 512], f16, tag="p",
                                  name=f"p_{b}_{qh}_{kc}")
                    s16_t = s16p.tile([128, 2, HPC, 512], f16, tag="s16",
                                      name=f"sc_{b}_{qh}_{kc}")
                    sps_l = []
                    for qq in range(2):
                        sps = psum.tile([128, HPC, 512], f32, tag="s",
                                        name=f"s_{b}_{qh}_{kc}_{qq}")
                        sps_l.append(sps)
                        q0 = qh * 1024 + qq * 512
                        for h in range(HPC):
                            nc.tensor.matmul(
                                sps[:, h, :],
                                KT[b][h * 64:(h + 1) * 64, ds(kc * 128, 128)],
                                QT[b][h * 64:(h + 1) * 64, ds(q0, 512)],
                                start=True, stop=True,
                            )
                    if attv_prev[0] is not None:
                        attv_prev[0]()
                    for qq in range(2):
                        sps = sps_l[qq]
                        nc.scalar.activation(p_t[:, qq, :, :], sps[:], EXP)
                        bal_copy(s16_t[:, qq, :, :], sps[:])
                        nc.vector.tensor_mul(
                            p_t[:, qq, :, :], p_t[:, qq, :, :], en_t[:, qq, :, :])
                    attv_prev[0] = mk_attv(kc, p_t)
                    nc.sync.dma_start(
                        s16_d.ap()[b, ds(kc * 128, 128), ds(qh * 2, 2), :, :],
                        s16_t[:])
                    if background:
                        p = background.pop(0)
                        if p is not None:
                            p()
                attv_prev[0]()
                # free the U psum slots ASAP: copy U to SBUF, then normalize
                # from SBUF (h0 has U in rows 0:64, Z in 64:128; h1 mirrored).
                ucp = []
                for h in range(HPC):
                    u_sb = rp.tile([128, 1024], f32, tag=f"ucp{h}",
                                   name=f"ucp{h}_{b}_{qh}")
                    nc.vector.tensor_copy(u_sb[:], u_ps[h][:])
                    ucp.append(u_sb)
                z0s = rp.tile([64, 1024], f32, tag="z0s", name=f"z0s_{b}_{qh}")
                nc.sync.dma_start(z0s[:, :], ucp[0][64:128, :])
                r0 = rp.tile([64, 1024], f32, tag="r0", name=f"r0_{b}_{qh}")
                nc.vector.reciprocal_approx_fast(r0[:, :], z0s[:, :])
                nc.vector.tensor_mul(
                    OT[b][0:64, ds(qh * 1024, 1024)], ucp[0][0:64, :], r0[:, :])
                r1 = rp.tile([64, 1024], f32, tag="r1", name=f"r1_{b}_{qh}")
                nc.vector.reciprocal_approx_fast(r1[:, :], ucp[1][0:64, :])
                r1s = rp.tile([128, 1024], f32, tag="r1s", name=f"r1s_{b}_{qh}")
                nc.sync.dma_start(r1s[64:128, :], r1[:, :])
                nc.vector.tensor_mul(
                    OT[b][64:128, ds(qh * 1024, 1024)], ucp[1][64:128, :],
                    r1s[64:128, :])

        # ---- schedule: qkv(b0); att(b0) pumping qkv(b1); att(b1) pumping final(b0);
        # ---- then final(b1).
        for piece in qkv_pieces(0):
            piece()
        bg = list(qkv_pieces(1))
        bg[0]()   # start xT(b1) loads right away (DMA only)
        bg = [None] * 8 + bg[1:]
        run_att(0, bg)
        for piece in bg:
            piece()
        bg = list(final_pieces(0))
        run_att(1, bg)
        for piece in bg:
            piece()
        for piece in final_pieces(1):
            piece()

    nc.compile()
    return nc


def _get_program():
    global _PROGRAM
    if _PROGRAM is None:
        _PROGRAM = _build_program()
    return _PROGRAM


def _host_noise_en(q_mask):
    """en = exp(noise + maskbias) in [B, h, q, k] layout, float32."""
    import jax
    import jax.numpy as jnp

    cpu = jax.local_devices(backend="cpu")[0]
    with jax.default_device(cpu):
        u = jax.random.uniform(jax.random.key(42), (B, NH, S, S), dtype=jnp.float32)
        u = np.asarray(u)
    noise = np.float32(1.0 / WEIBULL_K) * np.log(
        -np.log(u * np.float32(1.0 - 2.0 * EPS) + np.float32(EPS)) + np.float32(EPS))
    del u
    qm = np.asarray(q_mask, dtype=np.float32)
    bias = np.float32(-100000.0) * (1.0 - qm[:, :, None] * qm[:, None, :])  # [B,q,k]
    noise += bias[:, None, :, :]
    np.exp(noise, out=noise)
    return noise  # now en


def run(inputs, trace=False, trace_cores=None):
    from concourse.bass_utils import run_bass_kernel_spmd

    x = np.asarray(inputs["x"], dtype=np.float32)
    q_mask = np.asarray(inputs["q_mask"], dtype=np.float32)
    Wq = np.asarray(inputs["Wq"], dtype=np.float32)
    Wk = np.asarray(inputs["Wk"], dtype=np.float32)
    Wv = np.asarray(inputs["Wv"], dtype=np.float32)
    Wo = np.asarray(inputs["Wo"], dtype=np.float32)

    nc = _get_program()
    en_full = _host_noise_en(q_mask)  # [B, NH, q, k] f32

    xT16 = np.ascontiguousarray(x.transpose(0, 2, 1)).astype(np.float16)  # [B,H,S]
    scale = np.float32(1.0 / np.sqrt(HD))
    wq_s = (Wq * scale).astype(np.float16)
    wk_s = Wk.astype(np.float16)
    wv_s = Wv.astype(np.float16)
    wo_s = Wo.astype(np.float16)

    in_maps = []
    for c in range(NCORES):
        cols = slice(c * 128, (c + 1) * 128)
        # en layout: [b, j(2), q, k] -> [b, k, qQ, j, qr]
        en_c = en_full[:, 2 * c:2 * c + 2].reshape(B, HPC, NQQ, 512, S)
        en_c = np.ascontiguousarray(en_c.transpose(0, 4, 2, 1, 3)).astype(np.float16)
        in_maps.append({
            "xT": xT16,
            "wq": np.ascontiguousarray(wq_s[:, cols].reshape(8, 128, 128)),
            "wk": np.ascontiguousarray(wk_s[:, cols].reshape(8, 128, 128)),
            "wv": np.ascontiguousarray(wv_s[:, cols].reshape(8, 128, 128)),
            "wo": np.ascontiguousarray(wo_s[cols, :]),
            "en": en_c,
        })
    del en_full

    res = run_bass_kernel_spmd(
        nc, in_maps, core_ids=list(range(NCORES)), trace=trace,
        trace_cores=trace_cores)

    out = np.zeros((B, S, H), dtype=np.float32)
    logits_raw = np.empty((B, NH, S, S), dtype=np.float32)
    for c in range(NCORES):
        r = res.results[c]
        out += r["outp"].astype(np.float32)
        # s16 layout [b, k, qQ, j, qr] -> logits_raw[b, 2c+j, q, k]
        lr = r["s16"].transpose(0, 3, 2, 4, 1)  # [b, j, qQ, qr, k]
        logits_raw[:, 2 * c:2 * c + 2] = lr.reshape(B, HPC, S, S).astype(np.float32)
    return (out, logits_raw), res


def kernel(**inputs):
    (out, logits_raw), _ = run(inputs, trace=False)
    return out, logits_raw


# revision 6
# speedup vs baseline: 1.1502x; 1.1502x over previous
"""Trainium2 Bass kernel for noisy-softmax multi-head attention (nn_BAttention).

Full inputs -> full outputs; internally shards (batch x head) across 8 NeuronCores
Megatron-style (2 heads per core, Q/K/V/O weights column/row-sharded).

Math: reference computes
    q = split(x@Wq)/sqrt(hd); k = split(x@Wk); v = split(x@Wv)
    logits_raw = q @ k^T            (per head)         <- output 2
    att = softmax(logits_raw + maskbias - gammaln + noise)
    out = merge(att @ v) @ Wo                          <- output 1
The gammaln shift is softmax-invariant. noise depends only on key(42), so the
host precomputes en = exp(noise + maskbias); the device computes
    P = exp(S) * en;  out_h = (P^T V)/Z with Z = sum_k P  (via ones columns).
Everything runs transposed (S^T = K^T-stationary x Q^T-moving) so no on-chip
transposes are needed except V (done on the PE).
"""
import os
import sys

import numpy as np

for _p in ("/opt/trn_rl_repo",):
    if os.path.isdir(_p) and _p not in sys.path:
        sys.path.append(_p)

B, S, H = 2, 2048, 1024
NH, HD = 16, 64
NCORES = 8
HPC = NH // NCORES          # heads per core
WEIBULL_K = 10.0
EPS = 1e-5
NKC = S // 128              # 16 k-strips
NQQ = S // 512              # 4 q-quarters

_PROGRAM = None


def _build_program():
    import concourse.bass as bass
    import concourse.bacc as bacc
    import concourse.mybir as mybir
    import concourse.tile as tile
    from concourse.masks import make_identity
    from contextlib import ExitStack

    f32, f16 = mybir.dt.float32, mybir.dt.float16
    EXP = mybir.ActivationFunctionType.Exp
    ds = bass.ds

    nc = bacc.Bacc("TRN2", target_bir_lowering=False, debug=False,
                   enable_asserts=True, num_devices=NCORES)

    xT_d = nc.dram_tensor("xT", [B, H, S], f16, kind="ExternalInput")
    wq_d = nc.dram_tensor("wq", [8, 128, 128], f16, kind="ExternalInput")
    wk_d = nc.dram_tensor("wk", [8, 128, 128], f16, kind="ExternalInput")
    wv_d = nc.dram_tensor("wv", [8, 128, 128], f16, kind="ExternalInput")
    wo_d = nc.dram_tensor("wo", [128, H], f16, kind="ExternalInput")
    # layouts: [b, k, qQ, h, qr] with q = qQ*512 + qr
    en_d = nc.dram_tensor("en", [B, S, NQQ, HPC, 512], f16, kind="ExternalInput")
    s16_d = nc.dram_tensor("s16", [B, S, NQQ, HPC, 512], f16, kind="ExternalOutput")
    outp_d = nc.dram_tensor("outp", [B, S, H], f16, kind="ExternalOutput")

    with tile.TileContext(nc) as tc, ExitStack() as ctx:
        const = ctx.enter_context(tc.tile_pool(name="const", bufs=1))
        wpool = ctx.enter_context(tc.tile_pool(name="wpool", bufs=1))
        xtp = ctx.enter_context(tc.tile_pool(name="xtp", bufs=1))
        qkv = ctx.enter_context(tc.tile_pool(name="qkv", bufs=1))
        vop = ctx.enter_context(tc.tile_pool(name="vop", bufs=2))
        enp = ctx.enter_context(tc.tile_pool(name="enp", bufs=6))
        pp = ctx.enter_context(tc.tile_pool(name="pp", bufs=4))
        s16p = ctx.enter_context(tc.tile_pool(name="s16p", bufs=6))
        rp = ctx.enter_context(tc.tile_pool(name="rp", bufs=2))
        otp = ctx.enter_context(tc.tile_pool(name="otp", bufs=1))
        osp = ctx.enter_context(tc.tile_pool(name="osp", bufs=3))
        psum = ctx.enter_context(tc.tile_pool(name="psum", bufs=2, space="PSUM"))

        copy_ctr = [0]

        def bal_copy(dst, srcap):
            # ~3/8 of PSUM->SBUF copies on ACT, rest on DVE (keeps both ~equal)
            i = copy_ctr[0]; copy_ctr[0] += 1
            if i % 8 < 3:
                nc.scalar.copy(dst, srcap)
            else:
                nc.vector.tensor_copy(dst, srcap)

        ident = const.tile([128, 128], f16, tag="ident")
        make_identity(nc, ident[:])

        w_sb = {}
        for nm, d in (("wq", wq_d), ("wk", wk_d), ("wv", wv_d)):
            t = wpool.tile([128, 8, 128], f16, tag=nm, name=nm + "_sb")
            for ch in range(8):
                nc.sync.dma_start(t[:, ch, :], d.ap()[ch])
            w_sb[nm] = t
        wo_sb = wpool.tile([128, H], f16, tag="wo")
        nc.sync.dma_start(wo_sb[:], wo_d.ap())

        QT, KT, OT = {}, {}, {}
        VO = {}  # VO[(b, h, kc)]
        for b in range(B):
            QT[b] = qkv.tile([128, S], f16, tag=f"qt{b}", name=f"qt{b}")
            KT[b] = qkv.tile([128, S], f16, tag=f"kt{b}", name=f"kt{b}")
            OT[b] = otp.tile([128, S], f16, tag=f"ot{b}", name=f"ot{b}")

        def qkv_pieces(b):
            """Generator of work-unit closures for batch-b QKV + V transpose."""
            xt = []

            def load_xt():
                for ch in range(8):
                    t = xtp.tile([128, S], f16, tag=f"xt{ch}", name=f"xt{b}_{ch}")
                    nc.sync.dma_start(t[:], xT_d.ap()[b, ds(ch * 128, 128), :])
                    xt.append(t)
            yield load_xt

            vt_box = []

            def proj(wname, dest, qd, destcol):
                def go():
                    ps = psum.tile([128, 1024], f32, tag="s", bufs=3,
                                   name=f"qkv_{b}_{wname}_{qd}")
                    for ch in range(8):
                        for nn in range(2):
                            nc.tensor.matmul(
                                ps[:, ds(nn * 512, 512)],
                                w_sb[wname][:, ch, :],
                                xt[ch][:, ds(qd * 1024 + nn * 512, 512)],
                                start=(ch == 0), stop=(ch == 7),
                            )
                    bal_copy(dest[:, ds(destcol, 1024)], ps[:])
                return go

            def mk_vt():
                vt = qkv.tile([128, S], f16, tag="vt", name=f"vt{b}", bufs=2)
                vt_box.append(vt)
            yield mk_vt
            for qd in range(2):
                yield proj("wq", QT[b], qd, qd * 1024)
            for qd in range(2):
                yield proj("wk", KT[b], qd, qd * 1024)

            def projv(qd):
                def go():
                    proj("wv", vt_box[0], qd, qd * 1024)()
                return go
            for qd in range(2):
                yield projv(qd)

            def vtrans(kc):
                def go():
                    vt = vt_box[0]
                    pst = psum.tile([128, 128], f16, tag="s", bufs=3, name=f"vtr{b}_{kc}")
                    nc.tensor.transpose(pst[:], vt[:, ds(kc * 128, 128)], ident[:])
                    vo0 = vop.tile([128, 128], f16, tag=f"vo0_{kc}",
                                   name=f"vo0_{b}_{kc}")
                    vo1 = vop.tile([128, 128], f16, tag=f"vo1_{kc}",
                                   name=f"vo1_{b}_{kc}")
                    nc.vector.tensor_copy(vo0[:, 0:64], pst[:, 0:64])
                    nc.gpsimd.memset(vo0[:, 64:128], 1.0)
                    nc.vector.tensor_copy(vo1[:, 64:128], pst[:, 64:128])
                    nc.gpsimd.memset(vo1[:, 0:64], 1.0)
                    VO[(b, 0, kc)] = vo0
                    VO[(b, 1, kc)] = vo1
                return go
            for kc in range(NKC):
                yield vtrans(kc)

        def final_pieces(b):
            """Generator of work-unit closures for batch-b output projection."""
            def fin(st):
                def go():
                    psf = psum.tile([128, 1024], f32, tag="s", bufs=3, name=f"fin_{b}_{st}")
                    for hh in range(2):
                        nc.tensor.matmul(
                            psf[:, ds(hh * 512, 512)],
                            OT[b][:, ds(st * 128, 128)],
                            wo_sb[:, ds(hh * 512, 512)],
                            start=True, stop=True,
                        )
                    ost = osp.tile([128, 1024], f16, tag="ost", name=f"ost_{b}_{st}")
                    bal_copy(ost[:], psf[:])
                    nc.sync.dma_start(outp_d.ap()[b, ds(st * 128, 128), :], ost[:])
                return go
            for st in range(S // 128):
                yield fin(st)

        def run_att(b, background):
            """Attention for batch b, one q-quarter (512 cols) at a time."""
            for qQ in range(NQQ):
                u_ps = [
                    psum.tile([128, 512], f32, tag="u", name=f"u{h}_{b}_{qQ}",
                              bufs=2)
                    for h in range(HPC)
                ]
                attv_prev = [None]
                en_pair = [None]
                s16_pair = [None]

                def mk_attv(kc, p_t):
                    def go():
                        for h in range(HPC):
                            nc.tensor.matmul(
                                u_ps[h][:, :],
                                VO[(b, h, kc)][:],
                                p_t[:, h, :],
                                start=(kc == 0), stop=(kc == NKC - 1),
                            )
                    return go

                for kc in range(NKC):
                    a = kc % 2
                    if a == 0:
                        en_t = enp.tile([128, 2, HPC, 512], f16, tag="en",
                                        name=f"en_{b}_{qQ}_{kc}")
                        nc.sync.dma_start(
                            en_t[:],
                            en_d.ap()[b, ds(kc * 128, 256), qQ, :, :]
                            .rearrange("(a p) h r -> p a h r", a=2))
                        en_pair[0] = en_t
                        s16_pair[0] = s16p.tile([128, 2, HPC, 512], f16,
                                                tag="s16",
                                                name=f"sc_{b}_{qQ}_{kc}")
                    sps = psum.tile([128, HPC, 512], f32, tag="s",
                                    name=f"s_{b}_{qQ}_{kc}", bufs=3)
                    for h in range(HPC):
                        nc.tensor.matmul(
                            sps[:, h, :],
                            KT[b][h * 64:(h + 1) * 64, ds(kc * 128, 128)],
                            QT[b][h * 64:(h + 1) * 64, ds(qQ * 512, 512)],
                            start=True, stop=True,
                        )
                    if attv_prev[0] is not None:
                        attv_prev[0]()
                    p_t = pp.tile([128, HPC, 512], f16, tag="p",
                                  name=f"p_{b}_{qQ}_{kc}")
                    nc.scalar.activation(p_t[:], sps[:], EXP)
                    bal_copy(s16_pair[0][:, a, :, :], sps[:])
                    nc.vector.tensor_mul(p_t[:], p_t[:], en_pair[0][:, a, :, :])
                    attv_prev[0] = mk_attv(kc, p_t)
                    if a == 1:
                        nc.sync.dma_start(
                            s16_d.ap()[b, ds((kc - 1) * 128, 256), qQ, :, :]
                            .rearrange("(a p) h r -> p a h r", a=2),
                            s16_pair[0][:])
                        if background:
                            p = background.pop(0)
                            if p is not None:
                                p()
                attv_prev[0]()
                # normalization: h0 has U in rows 0:64, Z in 64:128; h1 mirrored
                qs = ds(qQ * 512, 512)
                zc = rp.tile([128, 512], f32, tag="zc", name=f"zc_{b}_{qQ}")
                nc.vector.tensor_copy(zc[64:128, :], u_ps[0][64:128, :])
                z0s = rp.tile([64, 512], f32, tag="z0s", name=f"z0s_{b}_{qQ}")
                nc.sync.dma_start(z0s[:, :], zc[64:128, :])
                r0 = rp.tile([64, 512], f32, tag="r0", name=f"r0_{b}_{qQ}")
                nc.vector.reciprocal_approx_fast(r0[:, :], z0s[:, :])
                nc.vector.tensor_mul(OT[b][0:64, qs], u_ps[0][0:64, :], r0[:, :])
                r1 = rp.tile([64, 512], f32, tag="r1", name=f"r1_{b}_{qQ}")
                nc.vector.reciprocal_approx_fast(r1[:, :], u_ps[1][0:64, :])
                r1s = rp.tile([128, 512], f32, tag="r1s", name=f"r1s_{b}_{qQ}")
                nc.sync.dma_start(r1s[64:128, :], r1[:, :])
                nc.vector.tensor_mul(OT[b][64:128, qs], u_ps[1][64:128, :],
                                     r1s[64:128, :])

        # ---- schedule: qkv(b0); att(b0) pumping qkv(b1); att(b1) pumping final(b0);
        # ---- then final(b1).
        for piece in qkv_pieces(0):
            piece()
        bg = list(qkv_pieces(1))
        bg[0]()   # start xT(b1) loads right away (DMA only)
        bg = [None] * 4 + bg[1:]
        run_att(0, bg)
        for piece in bg:
            piece()
        bg = list(final_pieces(0))
        run_att(1, bg)
        for piece in bg:
            piece()
        for piece in final_pieces(1):
            piece()

    nc.compile()
    return nc


def _get_program():
    global _PROGRAM
    if _PROGRAM is None:
        _PROGRAM = _build_program()
    return _PROGRAM


def _host_noise_en(q_mask):
    """en = exp(noise + maskbias) in [B, h, q, k] layout, float32."""
    import jax
    import jax.numpy as jnp

    cpu = jax.local_devices(backend="cpu")[0]
    with jax.default_device(cpu):
        u = jax.random.uniform(jax.random.key(42), (B, NH, S, S), dtype=jnp.float32)
        u = np.asarray(u)
    noise = np.float32(1.0 / WEIBULL_K) * np.log(
        -np.log(u * np.float32(1.0 - 2.0 * EPS) + np.float32(EPS)) + np.float32(EPS))
    del u
    qm = np.asarray(q_mask, dtype=np.float32)
    bias = np.float32(-100000.0) * (1.0 - qm[:, :, None] * qm[:, None, :])  # [B,q,k]
    noise += bias[:, None, :, :]
    np.exp(noise, out=noise)
    return noise  # now en


def run(inputs, trace=False, trace_cores=None):
    from concourse.bass_utils import run_bass_kernel_spmd

    x = np.asarray(inputs["x"], dtype=np.float32)
    q_mask = np.asarray(inputs["q_mask"], dtype=np.float32)
    Wq = np.asarray(inputs["Wq"], dtype=np.float32)
    Wk = np.asarray(inputs["Wk"], dtype=np.float32)
    Wv = np.asarray(inputs["Wv"], dtype=np.float32)
    Wo = np.asarray(inputs["Wo"], dtype=np.float32)

    nc = _get_program()
    en_full = _host_noise_en(q_mask)  # [B, NH, q, k] f32

    xT16 = np.ascontiguousarray(x.transpose(0, 2, 1)).astype(np.float16)  # [B,H,S]
    scale = np.float32(1.0 / np.sqrt(HD))
    wq_s = (Wq * scale).astype(np.float16)
    wk_s = Wk.astype(np.float16)
    wv_s = Wv.astype(np.float16)
    wo_s = Wo.astype(np.float16)

    in_maps = []
    for c in range(NCORES):
        cols = slice(c * 128, (c + 1) * 128)
        # en layout: [b, j(2), q, k] -> [b, k, qQ, j, qr]
        en_c = en_full[:, 2 * c:2 * c + 2].reshape(B, HPC, NQQ, 512, S)
        en_c = np.ascontiguousarray(en_c.transpose(0, 4, 2, 1, 3)).astype(np.float16)
        in_maps.append({
            "xT": xT16,
            "wq": np.ascontiguousarray(wq_s[:, cols].reshape(8, 128, 128)),
            "wk": np.ascontiguousarray(wk_s[:, cols].reshape(8, 128, 128)),
            "wv": np.ascontiguousarray(wv_s[:, cols].reshape(8, 128, 128)),
            "wo": np.ascontiguousarray(wo_s[cols, :]),
            "en": en_c,
        })
    del en_full

    res = run_bass_kernel_spmd(
        nc, in_maps, core_ids=list(range(NCORES)), trace=trace,
        trace_cores=trace_cores)

    out = np.zeros((B, S, H), dtype=np.float32)
    logits_raw = np.empty((B, NH, S, S), dtype=np.float32)
    for c in range(NCORES):
        r = res.results[c]
        out += r["outp"].astype(np.float32)
        # s16 layout [b, k, qQ, j, qr] -> logits_raw[b, 2c+j, q, k]
        lr = r["s16"].transpose(0, 3, 2, 4, 1)  # [b, j, qQ, qr, k]
        logits_raw[:, 2 * c:2 * c + 2] = lr.reshape(B, HPC, S, S).astype(np.float32)
    return (out, logits_raw), res


def kernel(**inputs):
    (out, logits_raw), _ = run(inputs, trace=False)
    return out, logits_raw
